# revision 15
# baseline (speedup 1.0000x reference)
"""Trainium2 Bass kernel for grouped 128x128 sparse attention + output proj.

Problem (hardcoded): qkv [2, 65536, 576] f32, tk_id [2, 65536] int32 in [0,64),
proj_w [192,192], proj_b [192].  c=192, heads=6, dh=32, group size 128,
ng=512 per batch (no padding since 65536 % 128 == 0).

Strategy:
  - Host: stable argsort by cluster id, gather qkv, reshape to 1024 independent
    groups; data-parallel shard 128 groups/core across 8 NeuronCores.
  - Device (per group g, heads h):
      S^T_h = k_h q_h^T        (6 matmuls, K=dh=32, out [k_tok, q_tok] psum f32)
      P^T   = exp(S^T)         (ScalarE activation -> SBUF bf16, per pair)
      [attnout | denom] = P^T.T @ [v_h | 1]  (6 matmuls -> psum [q, 33])
      normalize by per-partition reciprocal of denom (DVE, broadcast AP)
      transpose attnout (PE transpose) -> attnout^T [cin, tok]
      proj: out = attnout^T.T @ W^T (+bias via ones-rows)  -> psum [tok, 192]
      copy -> SBUF -> DMA out f32
  - Host: gather per-core outputs, inverse permutation, return [2, 65536, 192].

All matmul operands live at SBUF base partition 0 (matmuls with mixed base
partitions crash this runtime stack).
"""

import numpy as np
import ml_dtypes

BF16 = ml_dtypes.bfloat16

B = 2
N = 65536
C = 192
H = 6
DH = 32
GS = 128
NG_TOTAL = B * (N // GS)  # 1024 groups
N_CORES = 8
G_PER_CORE = NG_TOTAL // N_CORES  # 128
PAIRS = G_PER_CORE // 2  # 64

_nc_cache = {}


def _build_nc(num_pairs=PAIRS):
    """Build the Bass/Tile graph for one core (SPMD across all 8)."""
    from contextlib import ExitStack

    import concourse.tile as tile
    from concourse import bacc, mybir

    bf = mybir.dt.bfloat16
    f32 = mybir.dt.float32
    EXPF = mybir.ActivationFunctionType.Exp

    nc = bacc.Bacc("TRN2", target_bir_lowering=False, debug=False)

    P = num_pairs
    # qks: [pair, dh=32, (q/k, head, j, token)] — q cols 0:1536, k cols 1536:3072
    qks_d = nc.declare_dram_parameter("qks", [P, 32, 3072], bf, isOutput=False)
    v1_d = nc.declare_dram_parameter("v1", [P, 128, 396], bf, isOutput=False)
    wt1_d = nc.declare_dram_parameter("wt1", [128, 192], bf, isOutput=False)
    wt2_d = nc.declare_dram_parameter("wt2", [66, 192], bf, isOutput=False)
    iden_d = nc.declare_dram_parameter("iden", [128, 128], bf, isOutput=False)
    out_d = nc.declare_dram_parameter("out", [2 * P, 128, 192], f32, isOutput=True)

    with tile.TileContext(nc) as tc, ExitStack() as ctx:
        consts = ctx.enter_context(tc.tile_pool(name="consts", bufs=1))
        wt1_sb = consts.tile([128, 192], bf)
        nc.sync.dma_start(out=wt1_sb[:], in_=wt1_d[:, :])
        wt2_sb = consts.tile([66, 192], bf)
        nc.sync.dma_start(out=wt2_sb[:], in_=wt2_d[:, :])
        iden_sb = consts.tile([128, 128], bf)
        nc.sync.dma_start(out=iden_sb[:], in_=iden_d[:, :])

        inp = ctx.enter_context(tc.tile_pool(name="inp", bufs=4))
        expp = ctx.enter_context(tc.tile_pool(name="exps", bufs=3))
        attp = ctx.enter_context(tc.tile_pool(name="attn", bufs=3))
        recp = ctx.enter_context(tc.tile_pool(name="rec", bufs=3))
        atp = ctx.enter_context(tc.tile_pool(name="aT", bufs=3))
        outp = ctx.enter_context(tc.tile_pool(name="outs", bufs=4))

        # PSUM: gt (scores, 3 banks) x2 + avpj (av/tp/proj shared, 1 bank) x2
        # = 8 banks exactly. The avpj regions (AV out, transpose out, proj out)
        # overlap in time-sequenced phases within one bank.
        gtp = ctx.enter_context(tc.tile_pool(name="gt", bufs=2, space="PSUM"))
        avp = ctx.enter_context(tc.tile_pool(name="av", bufs=2, space="PSUM"))

        for p in range(P):
            qk = inp.tile([32, 3072], bf, tag="qk")
            nc.sync.dma_start(out=qk[:], in_=qks_d[p])
            vv = inp.tile([128, 396], bf, tag="vv")
            nc.sync.dma_start(out=vv[:], in_=v1_d[p])

            # scores for both groups: col 768j + 128h
            gt = gtp.tile([128, 1536], f32)
            for j in range(2):
                for h in range(6):
                    off = 256 * h + 128 * j
                    nc.tensor.matmul(
                        gt[:, 768 * j + 128 * h : 768 * j + 128 * h + 128],
                        qk[:, 1536 + off : 1536 + off + 128],
                        qk[:, off : off + 128],
                        start=True,
                        stop=True,
                    )
            # exp (psum f32 -> sbuf bf16), both groups in one activation
            exps = expp.tile([128, 1536], bf)
            nc.scalar.activation(exps[:], gt[:], EXPF)

            # AV: [q, 33] = P^T.T @ [v|1]; col 33h+32 = softmax denominator
            avpj = avp.tile([128, 396], f32)
            for j in range(2):
                for h in range(6):
                    nc.tensor.matmul(
                        avpj[:, 198 * j + 33 * h : 198 * j + 33 * h + 33],
                        exps[:, 768 * j + 128 * h : 768 * j + 128 * h + 128],
                        vv[:, 198 * j + 33 * h : 198 * j + 33 * h + 33],
                        start=True,
                        stop=True,
                    )

            # normalize: reciprocal of denoms + broadcast multiply.
            # att layout per pair [128, 400]: group j at cols 200j:
            #   [200j : 200j+128]   = heads 0-3 values (contiguous, for transpose)
            #   [200j+128:200j+194] = heads 4-5 incl denom cols (-> exact 1.0)
            av4 = avpj[:].rearrange("p (j h x) -> p j h x", j=2, x=33)
            rec = recp.tile([128, 12], mybir.dt.float32)
            rec3 = rec[:].rearrange("p (j h) -> p j h", j=2)
            nc.vector.reciprocal(rec3[:, :, :, None], av4[:, :, :, 32:33])
            att = attp.tile([128, 400], bf)
            att_a = att[:].rearrange("p (j x) -> p j x", j=2)[
                :, :, 0:128
            ].rearrange("p j (h d) -> p j h d", d=32)  # [128, 2, 4, 32]
            nc.vector.tensor_mul(
                att_a,
                av4[:, :, 0:4, 0:32],
                rec3[:, :, 0:4, None].to_broadcast((128, 2, 4, 32)),
            )
            att_b = att[:].rearrange("p (j x) -> p j x", j=2)[
                :, :, 128:194
            ].rearrange("p j (h d) -> p j h d", d=33)  # [128, 2, 2, 33]
            nc.vector.tensor_mul(
                att_b,
                av4[:, :, 4:6, :],
                rec3[:, :, 4:6, None].to_broadcast((128, 2, 2, 33)),
            )

            # transposes into the (now consumed) avpj bank, bitcast as bf16.
            # tp/aT layout: [t1_j0 | t1_j1 | t2_j0 | t2_j1]
            tp = avpj[:, 0:256].bitcast(bf)  # [128, 512] bf16 view
            aT = atp.tile([128, 512], bf)
            for j in range(2):
                nc.tensor.transpose(
                    tp[:, 128 * j : 128 * j + 128],
                    att[:, 200 * j : 200 * j + 128],
                    iden_sb[:],
                )
                nc.tensor.transpose(
                    tp[0:66, 256 + 128 * j : 256 + 128 * j + 128],
                    att[:, 200 * j + 128 : 200 * j + 194],
                    iden_sb[:],
                )
            nc.vector.tensor_copy(aT[:, 0:256], tp[:, 0:256])
            nc.vector.tensor_copy(aT[0:66, 256:512], tp[0:66, 256:512])

            # proj into avpj cols 0:384 (transposes already consumed):
            # out[tok, cout] = aT1.T @ wt1 + aT2.T @ wt2 (bias via ones-rows)
            for j in range(2):
                pj = avpj[:, 192 * j : 192 * j + 192]
                nc.tensor.matmul(
                    pj,
                    aT[0:128, 128 * j : 128 * j + 128],
                    wt1_sb[:],
                    start=True,
                    stop=False,
                )
                nc.tensor.matmul(
                    pj,
                    aT[0:66, 256 + 128 * j : 256 + 128 * j + 128],
                    wt2_sb[:],
                    start=False,
                    stop=True,
                )
            ob = outp.tile([128, 384], f32)
            nc.vector.tensor_copy(ob[:], avpj[:, 0:384])
            nc.gpsimd.dma_start(
                out=out_d[2 * p : 2 * p + 2].transpose([1, 0, 2]),
                in_=ob[:].rearrange("p (j c) -> p j c", j=2),
            )

    nc.compile()
    return nc


def _host_prep(qkv, tk_id, proj_w, proj_b):
    """Sort/gather/layout on host. Returns (in_maps, sort_idx)."""
    qkv = np.asarray(qkv, dtype=np.float32)
    tk_id = np.asarray(tk_id)
    proj_w = np.asarray(proj_w, dtype=np.float32)
    proj_b = np.asarray(proj_b, dtype=np.float32)

    sort_idx = np.argsort(tk_id, axis=-1, kind="stable")  # [B, N]
    shuffled = np.take_along_axis(qkv, sort_idx[:, :, None], axis=1)  # [B,N,3C]

    y = shuffled.reshape(B, N // GS, GS, 3, H, DH)  # [b, ng, gs, 3, h, dh]
    y = y.reshape(NG_TOTAL, GS, 3, H, DH)  # [G, t, 3, h, dh]
    scale = DH ** (-0.5)
    q = y[:, :, 0] * scale  # [G, t, h, dh]
    k = y[:, :, 1]
    v = y[:, :, 2]

    # qs/ks: [pair, dh=32, (head, j, token)]
    def pack_qk(a):
        # a: [G, t, h, d] -> [G/2, d, h, j(2), t]
        g = a.shape[0]
        a = a.reshape(g // 2, 2, GS, H, DH)  # [p, j, t, h, d]
        a = a.transpose(0, 4, 3, 1, 2)  # [p, d, h, j, t]
        return np.ascontiguousarray(a).reshape(g // 2, DH, H * 2 * GS)

    qks = np.concatenate(
        [pack_qk(q), pack_qk(k)], axis=2
    ).astype(BF16)  # [512, 32, 3072]

    v1 = np.empty((NG_TOTAL, GS, H, DH + 1), dtype=np.float32)
    v1[..., :DH] = v
    v1[..., DH] = 1.0
    v1 = v1.reshape(NG_TOTAL, GS, H * (DH + 1))  # [G, 128, 198]
    g = v1.shape[0]
    v1p = (
        v1.reshape(g // 2, 2, GS, 198)
        .transpose(0, 2, 1, 3)
        .reshape(g // 2, GS, 396)
        .astype(BF16)
    )

    wt = proj_w.T.copy()  # [cin, cout]
    wt1 = wt[0:128].astype(BF16)
    half_b = 0.5 * proj_b
    wt2 = np.concatenate(
        [wt[128:160], half_b[None, :], wt[160:192], half_b[None, :]], axis=0
    ).astype(BF16)  # [66, 192]
    iden = np.eye(128, dtype=BF16)

    in_maps = []
    for core in range(N_CORES):
        s = slice(core * PAIRS, (core + 1) * PAIRS)
        in_maps.append(
            {
                "qks": np.ascontiguousarray(qks[s]),
                "v1": np.ascontiguousarray(v1p[s]),
                "wt1": wt1,
                "wt2": wt2,
                "iden": iden,
            }
        )
    return in_maps, sort_idx


def _host_unshard(results, sort_idx):
    out_sorted = np.concatenate(
        [np.asarray(r["out"], dtype=np.float32) for r in results], axis=0
    )  # [1024, 128, 192]
    out_sorted = out_sorted.reshape(B, N, C)
    final = np.empty_like(out_sorted)
    np.put_along_axis(final, sort_idx[:, :, None], out_sorted, axis=1)
    return final


def _get_nc():
    if "nc" not in _nc_cache:
        _nc_cache["nc"] = _build_nc()
    return _nc_cache["nc"]


def _run(in_maps, trace=False):
    from concourse import bass_utils

    nc = _get_nc()
    return bass_utils.run_bass_kernel_spmd(
        nc, in_maps, core_ids=list(range(N_CORES)), trace=trace
    )


def kernel(qkv, tk_id, x_size=None, proj_w=None, proj_b=None):
    in_maps, sort_idx = _host_prep(qkv, tk_id, proj_w, proj_b)
    res = _run(in_maps, trace=False)
    return _host_unshard(res.results, sort_idx)
